# revision 42
# baseline (speedup 1.0000x reference)
"""Differential attention (BiomedCLIP ViT-B) Bass kernel for 8 Trainium2 cores.

Strategy
--------
Data-parallel over batch: B=128 -> 16 batches per core, no collectives.

Host-side preprocessing folds the "differential" part away entirely:
  - lambda scalar lv is computed on host from lq1/lk1/lq2/lk2 (tiny tensors)
  - out = concat([x1 - lv*x2, x2]) @ Wp^T  ==  concat([x1, x2]) @ Wp_eff^T
    with Wp_eff = [Wp[:, :384], Wp[:, 384:] - lv*Wp[:, :384]]
  - attention scale 1/sqrt(hd) is folded into Wq/bq
so the device kernel is a standard 12-head MHA block.

Device-side layout: everything stays in "transposed" (feature-on-partition)
layout so no PE transposes are needed:
  Q^T = WqT.T @ xT           [o, t]   (o on partitions)
  K^T likewise               [o, t]
  V   = xT.T @ WvT (+ ones)  [t, o]   (natural; 65th column of ones per head)
  S^T = K_h @ Q_h^T          [tk, tq] per (batch, head)
  P^T = exp(S^T)             bf16
  U'^T = V'_h.T @ P^T        [65, tq] (row 64 = softmax denominators)
  O^T = U'^T[0:64] * (1/denom broadcast)
  Y   = O^T.T @ WpT_eff + bp [t, e]
"""

import sys
import os

sys.path.insert(0, "/opt/trn_rl_repo")

import numpy as np
import ml_dtypes

BF16 = ml_dtypes.bfloat16

# Problem constants (hardcoded per contract)
B, N, C = 128, 197, 768
NH, HD = 12, 64
NCORES = 8
BPC = B // NCORES          # batches per core = 16
T = BPC * N                # tokens per core = 3152
KC = C // 128              # contraction chunks = 6
NPAIR = NH // 2            # head pairs = 6
VW = NH * (HD + 1)         # V' width with ones columns = 780

_NC_CACHE = {}
LAST_RESULT = None         # BassKernelResults of the most recent run (for test.py)


def _build_nc(stage=4):
    """Build the Bass/Tile program (identical SPMD program for all 8 cores)."""
    import concourse.bass as bass
    from concourse import bacc, mybir
    from concourse.tile import TileContext
    from contextlib import ExitStack

    f32 = mybir.dt.float32
    bf16 = mybir.dt.bfloat16
    AF = mybir.ActivationFunctionType

    nc = bacc.Bacc(trn_type="TRN2", target_bir_lowering=False, debug=False)

    xT_d = nc.declare_dram_parameter("xT", [C, T], bf16, isOutput=False)
    wq_d = nc.declare_dram_parameter("wqT", [C, C], bf16, isOutput=False)
    wk_d = nc.declare_dram_parameter("wkT", [C, C], bf16, isOutput=False)
    wv_d = nc.declare_dram_parameter("wvT", [C, C], bf16, isOutput=False)
    wp_d = nc.declare_dram_parameter("wpT", [C, C], bf16, isOutput=False)
    bq_d = nc.declare_dram_parameter("bq2", [128, KC], f32, isOutput=False)
    bk_d = nc.declare_dram_parameter("bk2", [128, KC], f32, isOutput=False)
    bv_d = nc.declare_dram_parameter("bv", [C], f32, isOutput=False)
    bp_d = nc.declare_dram_parameter("bp2", [128, KC], f32, isOutput=False)
    y_d = nc.declare_dram_parameter("y", [C, T], bf16, isOutput=True)

    with TileContext(nc) as tc, ExitStack() as ctx:
        consts = ctx.enter_context(tc.tile_pool(name="consts", bufs=1))
        xt_pool = ctx.enter_context(tc.tile_pool(name="xt", bufs=2))
        qk_pool = ctx.enter_context(tc.tile_pool(name="qk", bufs=2))
        ut_pool = ctx.enter_context(tc.tile_pool(name="ut", bufs=2))
        p_pool = ctx.enter_context(tc.tile_pool(name="pp", bufs=6))
        r_pool = ctx.enter_context(tc.tile_pool(name="rr", bufs=4))
        y_pool = ctx.enter_context(tc.tile_pool(name="yy", bufs=3))
        psum = ctx.enter_context(tc.tile_pool(name="psum", bufs=2, space="PSUM"))

        # --- constants ---
        # DMA issue order/queues tuned so the first Q-proj matmul can start
        # ~2us in: x tiles for pair 0 go on the sync queue, wq on scalar,
        # wk on vector, wv/wp+biases on gpsimd (idle at startup).
        xt_tiles = {}

        def issue_xt(p, eng=None, split=1):
            # one strided 3D DMA per pair (chunk k -> columns k*394..):
            # 650ns/trigger makes per-chunk DMAs a startup bottleneck
            t0 = p * (2 * N)
            t_ = xt_pool.tile([128, KC * 2 * N], bf16, tag="xt",
                              name=f"xt_{p}")
            t3 = t_.rearrange("p (k c) -> p k c", c=2 * N)
            xa = xT_d[:, :]
            kh = KC // split
            for s in range(split):
                (eng or nc.gpsimd).dma_start(
                    out=t3[:, s * kh:(s + 1) * kh, :],
                    in_=bass.AP(tensor=xa.tensor, offset=s * kh * 128 * T + t0,
                                ap=[[T, 128], [128 * T, kh], [1, 2 * N]]))
            xt_tiles[p] = [t_[:, k * 2 * N:(k + 1) * 2 * N] for k in range(KC)]

        # startup: wv chunks split across scalar+gpsimd rings (V-proj runs
        # first), xt pair 0 on sync in parallel; wq/wk interleaved next so
        # the Q/K projections never wait; wp + biases trail.
        def load_w(d, tag, engines):
            # 3 chunks per ring: two parallel strided DMAs instead of six
            t_ = consts.tile([128, KC * C], bf16, tag=tag, name=tag)
            t3 = t_.rearrange("p (k c) -> p k c", c=C)
            da = d[:, :]
            kh = KC // len(engines)
            for s, eng in enumerate(engines):
                eng.dma_start(
                    out=t3[:, s * kh:(s + 1) * kh, :],
                    in_=bass.AP(tensor=da.tensor, offset=s * kh * 128 * C,
                                ap=[[C, 128], [128 * C, kh], [1, C]]))
            return [t_[:, k * C:(k + 1) * C] for k in range(KC)]

        wv = load_w(wv_d, "wv", (nc.scalar, nc.gpsimd))
        issue_xt(0, eng=nc.sync, split=2)
        bvb = consts.tile([128, C], f32, tag="bvb")
        bv_ap = bv_d[:]
        nc.gpsimd.dma_start(
            out=bvb, in_=bass.AP(tensor=bv_ap.tensor, offset=0, ap=[[0, 128], [1, C]])
        )
        wq = load_w(wq_d, "wq", (nc.scalar, nc.gpsimd))
        bq2 = consts.tile([128, KC], f32, tag="bq2")
        nc.scalar.dma_start(out=bq2, in_=bq_d[:, :])
        wk = load_w(wk_d, "wk", (nc.scalar, nc.gpsimd))
        bk2 = consts.tile([128, KC], f32, tag="bk2")
        nc.scalar.dma_start(out=bk2, in_=bk_d[:, :])
        wp = load_w(wp_d, "wp", (nc.gpsimd,))
        bp2 = consts.tile([128, KC], f32, tag="bp2")
        nc.scalar.dma_start(out=bp2, in_=bp_d[:, :])
        zeros = consts.tile([128, 1], f32, tag="zeros")
        nc.vector.memset(zeros, 0.0)
        # persistent V'' tiles: ones initialized once at startup (columns
        # 0:64 per head stay 1.0 forever; V values land in 64:128)
        vts = {}
        for i in range(2):
            for b01 in range(2):
                for ts_ in range(2):
                    t_ = consts.tile([128, NH * 128], bf16,
                                     tag=f"vt{i}{b01}{ts_}", name=f"vt{i}{b01}{ts_}")
                    nc.vector.memset(t_, 1.0)
                    vts[(i, b01, ts_)] = t_

        BANK = 512  # fp32 elements per PSUM bank

        # --- phase bodies (emitted in software-pipelined order below) ---
        def proj_phase(p, qk_first=False):
            """Q^T/K^T/V'' projections for pair p. Independent of pair p-1's
            attention results, so it is emitted between attn(p-1) and
            out(p-1) to cover the trailing softmax-normalize latency."""
            xt = xt_tiles.pop(p)
            if qk_first:  # pair 0: DVE is idle, and wq/wk DMAs land first
                qkv = emit_qk(xt)
            # V'' projection first: its DVE bias-adds then drain during the
            # Q/K matmuls instead of gating out(p-1)'s first PSUM slot.
            # (natural layout; per head 128 cols: 64 ones + 64 V so AV puts
            # softmax denominators on PSUM partitions 0:64)
            vp = {}
            for b01 in range(2):
                for ts in range(2):
                    tsz = 128 if ts == 0 else N - 128
                    vt = vts[(p % 2, b01, ts)]
                    vt3 = vt.rearrange("p (h c) -> p h c", c=128)
                    tc0 = b01 * N + ts * 128
                    for oh in range(2):
                        ps = psum.tile([128, 384], f32, tag="proj")
                        for k in range(KC):
                            nc.tensor.matmul(
                                ps[:tsz],
                                lhsT=xt[k][:, tc0:tc0 + tsz],
                                rhs=wv[k][:, oh * 384:(oh + 1) * 384],
                                start=(k == 0), stop=(k == KC - 1),
                            )
                        nc.vector.tensor_add(
                            out=vt3[:tsz, oh * 6:(oh + 1) * 6, HD:128],
                            in0=ps[:tsz].rearrange("p (h d) -> p h d", d=HD),
                            in1=bvb[:tsz].rearrange("p (h d) -> p h d", d=HD)[
                                :, oh * 6:(oh + 1) * 6, :],
                        )
                    vp[(b01, ts)] = vt

            if qk_first:
                qt, kt = qkv
            else:
                qt, kt = emit_qk(xt)
            return qt, kt, vp

        def emit_qk(xt):
            qt = []
            kt = []
            for wts, bias_t, out_list, nm in ((wq, bq2, qt, "qt"), (wk, bk2, kt, "kt")):
                for j in range(KC):
                    ps = psum.tile([128, 2 * N], f32, tag="proj")
                    for k in range(KC):
                        nc.tensor.matmul(
                            ps, lhsT=wts[k][:, j * 128:(j + 1) * 128], rhs=xt[k],
                            start=(k == 0), stop=(k == KC - 1),
                        )
                    sb = qk_pool.tile([128, 2 * N], bf16, tag=f"{nm}{j}")
                    nc.scalar.activation(sb, ps, AF.Identity, bias=bias_t[:, j:j + 1])
                    out_list.append(sb)
            return qt, kt

        def attn_phase(p, qt, kt, vp, only_b01=None, ut=None, fillers=None):
            # attention per batch (software-pipelined across head pairs:
            # PE stream is S(0), S(1), AV(0), S(2), AV(1), ... so AV never
            # waits on the ACT exp of its own head pair). `fillers` are
            # independent PE work emitters interleaved between head pairs.
            fillers = list(fillers or [])
            if ut is None:
                ut = []
                for j in range(KC):
                    ut.append(ut_pool.tile([128, 2 * N], bf16, tag=f"ut{j}",
                                           name=f"ut{j}_{p}"))
            for b01 in (range(2) if only_b01 is None else [only_b01]):
                bcol = b01 * N

                pts = {}

                def emit_s(hp):
                    for tkc in range(2):
                        tksz = 128 if tkc == 0 else N - 128
                        tks0 = bcol + tkc * 128
                        # two banks in one tile: head even -> bank 0,
                        # head odd (weights at partition base 64, row group 1)
                        # -> bank 1. Separate accumulation groups.
                        sps = psum.tile([128, 2 * BANK], f32, tag="s")
                        for h01 in range(2):
                            nc.tensor.matmul(
                                sps[:tksz, h01 * BANK:h01 * BANK + N],
                                lhsT=kt[hp][h01 * HD:(h01 + 1) * HD,
                                            tks0:tks0 + tksz],
                                rhs=qt[hp][h01 * HD:(h01 + 1) * HD,
                                           bcol:bcol + N],
                                start=True, stop=True,
                            )
                        pt = p_pool.tile([128, 2 * N], bf16, tag="pt")
                        nc.scalar.activation(
                            pt[:tksz].rearrange("p (two x) -> p two x", x=N),
                            sps[:tksz].rearrange("p (two x) -> p two x",
                                                 x=BANK)[:, :, 0:N],
                            AF.Exp, bias=zeros[:tksz])
                        pts[(hp, tkc)] = pt

                def emit_av(hp):
                    ups = psum.tile([128, 2 * N], f32, tag="u")
                    nmm = 0
                    for h01 in range(2):
                        h = hp * 2 + h01
                        for tkc in range(2):
                            tksz = 128 if tkc == 0 else N - 128
                            nc.tensor.matmul(
                                ups[:, h01 * N:(h01 + 1) * N],
                                lhsT=vp[(b01, tkc)][:tksz, h * 128:(h + 1) * 128],
                                rhs=pts[(hp, tkc)][:tksz, h01 * N:(h01 + 1) * N],
                                start=(nmm == 0), stop=(nmm == 3),
                            )
                            nmm += 1
                    # rows 0:64 hold the replicated softmax denominators d
                    # (V' has its ones columns first; custom-DVE ops require
                    # partition offset 0). Single custom-DVE reciprocal keeps
                    # the ACT engine on the exp table — no table reloads.
                    rsb = r_pool.tile([HD, 2 * N], f32, tag="rsb")
                    nc.vector.reciprocal_approx_fast(out=rsb, in_=ups[0:HD, :])
                    for h01 in range(2):
                        r0 = h01 * HD
                        nc.vector.tensor_mul(
                            out=ut[hp][r0:r0 + HD, bcol:bcol + N],
                            in0=ups[HD:128, h01 * N:(h01 + 1) * N],
                            in1=rsb[:, h01 * N:(h01 + 1) * N],
                        )

                emit_s(0)
                for hp in range(1, NPAIR):
                    emit_s(hp)
                    emit_av(hp - 1)
                    if fillers:
                        fillers.pop(0)()
                emit_av(NPAIR - 1)
                while fillers:
                    fillers.pop(0)()
            return ut

        def out_group(p, ut, cols, j2):
            c0, c1 = cols
            t0 = p * (2 * N) + c0
            ps = psum.tile([128, 2 * N], f32, tag="proj")
            for j in range(KC):
                nc.tensor.matmul(
                    ps[:, 0:c1 - c0],
                    lhsT=wp[j][:, j2 * 128:(j2 + 1) * 128],
                    rhs=ut[j][:, c0:c1],
                    start=(j == 0), stop=(j == KC - 1),
                )
            ysbT = y_pool.tile([128, 2 * N], bf16, tag="y", bufs=6)
            nc.scalar.activation(ysbT[:, 0:c1 - c0], ps[:, 0:c1 - c0],
                                 AF.Identity, bias=bp2[:, j2:j2 + 1])
            nc.sync.dma_start(
                out=y_d[j2 * 128:(j2 + 1) * 128, t0:t0 + (c1 - c0)],
                in_=ysbT[:, 0:c1 - c0])

        def out_phase(p, ut, cols=(0, 2 * N), j2s=range(KC)):
            # transposed output projection Y^T = Wp_eff @ O^T: features on
            # partitions so the 394-token window fills the free dim exactly
            # and the bias is per-partition (ACT engine, exp-table friendly).
            # Host transposes Y^T back.
            for j2 in j2s:
                out_group(p, ut, cols, j2)

        # --- software-pipelined main loop over pairs of batches ---
        # PE order: proj(0), attn(0), proj(1), out(0), attn(1), proj(2),
        # out(1), ... so pair p's trailing normalize is covered by pair
        # p+1's projections before out(p) consumes ut.
        NP = BPC // 2
        issue_xt(1, eng=nc.sync)
        qkv = proj_phase(0)
        ut_prev = None
        for p in range(NP - 1):
            ut = attn_phase(p, *qkv)
            if p + 2 < NP:
                issue_xt(p + 2)
            qkv = proj_phase(p + 1)
            # defer the tail of out(NP-2) into the last pair's b0 attention
            out_phase(p, ut, j2s=range(KC) if p < NP - 2 else range(3))
            ut_prev = ut
        # last pair: no proj to overlap, so interleave deferred/own output-
        # projection groups into the attention streams to cover the
        # exp/normalize latencies
        p = NP - 1
        f0 = [lambda j2=j2: out_group(NP - 2, ut_prev, (0, 2 * N), j2)
              for j2 in range(3, KC)]
        ut = attn_phase(p, *qkv, only_b01=0, fillers=f0)
        f1 = [lambda j2=j2: out_group(p, ut, (0, N), j2)
              for j2 in range(KC)]
        attn_phase(p, *qkv, only_b01=1, ut=ut, fillers=f1)
        out_phase(p, ut, cols=(N, 2 * N))

    nc.finalize()
    return nc


def _prep_inputs(x, Wq, bq, Wk, bk, Wv, bv, Wp, bp, lq1, lk1, lq2, lk2,
                 lambda_init):
    """Host-side preprocessing -> per-core input maps."""
    f32 = np.float32
    x = np.asarray(x, f32)
    Wq = np.asarray(Wq, f32); bq = np.asarray(bq, f32)
    Wk = np.asarray(Wk, f32); bk = np.asarray(bk, f32)
    Wv = np.asarray(Wv, f32); bv = np.asarray(bv, f32)
    Wp = np.asarray(Wp, f32); bp = np.asarray(bp, f32)

    # lambda scalar (float32 math like the jax reference)
    l1 = np.exp(np.minimum((np.asarray(lq1, f32) * np.asarray(lk1, f32)).sum(axis=(-1, -2)), f32(5.0)))
    l2 = np.exp(np.minimum((np.asarray(lq2, f32) * np.asarray(lk2, f32)).sum(axis=(-1, -2)), f32(5.0)))
    lv = f32((l1 - l2 + f32(lambda_init)).mean())

    scale = f32(HD ** -0.5)
    wqT = np.ascontiguousarray((Wq * scale).T).astype(BF16)
    wkT = np.ascontiguousarray(Wk.T).astype(BF16)
    wvT = np.ascontiguousarray(Wv.T).astype(BF16)
    Wp_eff = np.concatenate([Wp[:, :C // 2], Wp[:, C // 2:] - lv * Wp[:, :C // 2]],
                            axis=1)
    wpT = np.ascontiguousarray(Wp_eff.T).astype(BF16)

    bq2 = np.ascontiguousarray((bq * scale).reshape(KC, 128).T).astype(f32)
    bk2 = np.ascontiguousarray(bk.reshape(KC, 128).T).astype(f32)
    bp2 = np.ascontiguousarray(bp.reshape(KC, 128).T).astype(f32)

    in_maps = []
    for c in range(NCORES):
        xc = x[c * BPC:(c + 1) * BPC].reshape(T, C)
        xT = np.ascontiguousarray(xc.T).astype(BF16)
        in_maps.append({
            "xT": xT, "wqT": wqT, "wkT": wkT, "wvT": wvT, "wpT": wpT,
            "bq2": bq2, "bk2": bk2, "bv": bv, "bp2": bp2,
        })
    return in_maps


def kernel(x, Wq, bq, Wk, bk, Wv, bv, Wp, bp, lq1, lk1, lq2, lk2,
           num_heads, lambda_init):
    global LAST_RESULT
    from concourse.bass_utils import run_bass_kernel_spmd

    assert int(num_heads) == NH
    assert tuple(np.asarray(x).shape) == (B, N, C)

    if "nc" not in _NC_CACHE:
        _NC_CACHE["nc"] = _build_nc()
    nc = _NC_CACHE["nc"]

    in_maps = _prep_inputs(x, Wq, bq, Wk, bk, Wv, bv, Wp, bp,
                           lq1, lk1, lq2, lk2, lambda_init)
    res = run_bass_kernel_spmd(nc, in_maps, list(range(NCORES)))
    LAST_RESULT = res
    out = np.concatenate(
        [res.results[c]["y"].astype(np.float32).T.reshape(BPC, N, C)
         for c in range(NCORES)], axis=0
    )
    return np.ascontiguousarray(out.astype(np.float32))

